# revision 84
# baseline (speedup 1.0000x reference)
"""Trainium2 Bass kernel for nn_AdvancedMixConsole (B=4, T=16, S=131072).

Sharding: 64 channels over 8 cores (8 ch/core), pan+track-sum on host.
Per channel on device (vs. the v1 kernel: skewed 8-deep software pipeline,
bf16 tensor-engine path, batched const DMAs, scan2-with-initial chaining,
row alpha-product via activation accum, engine-balanced elementwise):
  - 6-biquad cascade as one 12-state LTI block filter: y = Toeplitz(h) @
    x_chunks + U @ s_in; chunk incoming states from a 3-level hierarchical
    linear-recurrence solve on the tensor engine (fp32), x/h/U in bf16.
  - compressor soft-knee gain g on Act/DVE/Pool.
  - attack/release envelope: NPASS rounds of policy iteration; per round
    m -> alpha(+row-sum) -> scan(init 0) -> cross-row chain -> scan(init).
  - z = y * exp(mk' - LN10_20*e) -> DMA out (bf16).
"""
import os
import sys

import numpy as np

for _p in ("/opt/trn_rl_repo", os.path.expanduser("~/.axon_site/_ro/trn_rl_repo")):
    if os.path.isdir(_p) and _p not in sys.path:
        sys.path.insert(0, _p)

SR = 44100.0
EPS = 1e-8
S = 131072
L = 128
K = 1024
CH = 8           # channels per core
NCORES = 8
NPASS = int(os.environ.get("KNPASS", "2"))
TOEP_BF = os.environ.get("KMMBF", "bf") == "bf"  # bf16 Toeplitz matmuls
PEMODE = os.environ.get("KPEMODE", "mcnt")  # mcnt | accum | mscan
PE_ACCUM = PEMODE == "accum"
PE_MCNT = PEMODE == "mcnt"
LN10_20 = np.log(10.0) / 20.0
C1 = 20.0 / np.log(10.0)

NC128 = 140   # hmov(128) + wt(12) columns per channel in the [128,*] bf16 pack
NC12 = 1824   # g2t,g3t(768*2) + pd1t,pd2t,pd3t(96*3) in the [12,*] f32 pack


def _dn(p, i, lo, hi):
    return (p[..., i].astype(np.float64) * (hi - lo) + lo).reshape(-1)


def _biquad_coeffs(gain_db, cutoff, q, ftype):
    A = np.power(10.0, gain_db / 40.0)
    w0 = 2.0 * np.pi * (cutoff / SR)
    cw, sw = np.cos(w0), np.sin(w0)
    alpha = sw / (2.0 * q)
    sA = np.sqrt(A)
    if ftype == "low_shelf":
        b0 = A * ((A + 1) - (A - 1) * cw + 2 * sA * alpha)
        b1 = 2 * A * ((A - 1) - (A + 1) * cw)
        b2 = A * ((A + 1) - (A - 1) * cw - 2 * sA * alpha)
        a0 = (A + 1) + (A - 1) * cw + 2 * sA * alpha
        a1 = -2 * ((A - 1) + (A + 1) * cw)
        a2 = (A + 1) + (A - 1) * cw - 2 * sA * alpha
    elif ftype == "high_shelf":
        b0 = A * ((A + 1) + (A - 1) * cw + 2 * sA * alpha)
        b1 = -2 * A * ((A - 1) + (A + 1) * cw)
        b2 = A * ((A + 1) + (A - 1) * cw - 2 * sA * alpha)
        a0 = (A + 1) - (A - 1) * cw + 2 * sA * alpha
        a1 = 2 * ((A - 1) - (A + 1) * cw)
        a2 = (A + 1) - (A - 1) * cw - 2 * sA * alpha
    else:
        b0 = 1 + alpha * A
        b1 = -2 * cw
        b2 = 1 - alpha * A
        a0 = 1 + alpha / A
        a1 = -2 * cw
        a2 = 1 - alpha / A
    b = np.stack([b0, b1, b2], -1) / a0[..., None]
    a = np.stack([a0, a1, a2], -1) / a0[..., None]
    return b, a


def _host_constants(mix_params):
    p = mix_params
    N = p.shape[0] * p.shape[1]
    nyq = SR // 2 - 1000.0
    gain = np.power(10.0, _dn(p, 0, -24.0, 24.0) / 20.0)
    specs = [
        ("low_shelf", _dn(p, 1, -24, 24), _dn(p, 2, 20, 2000), _dn(p, 3, 0.1, 5.0)),
        ("peak", _dn(p, 4, -24, 24), _dn(p, 5, 80, 2000), _dn(p, 6, 0.1, 5.0)),
        ("peak", _dn(p, 7, -24, 24), _dn(p, 8, 2000, 8000), _dn(p, 9, 0.1, 5.0)),
        ("peak", _dn(p, 10, -24, 24), _dn(p, 11, 8000, 12000), _dn(p, 12, 0.1, 5.0)),
        ("peak", _dn(p, 13, -24, 24), _dn(p, 14, 12000, nyq), _dn(p, 15, 0.1, 5.0)),
        ("high_shelf", _dn(p, 16, -24, 24), _dn(p, 17, 6000, nyq), _dn(p, 18, 0.1, 5.0)),
    ]
    coeffs = [_biquad_coeffs(g, f, q, ft) for ft, g, f, q in specs]

    Ab = np.zeros((N, 12, 12))
    Bb = np.zeros((N, 12))
    ux = np.ones((N, 7))
    us = np.zeros((N, 7, 12))
    Ai = np.zeros((N, 6, 2, 2))
    Bi = np.zeros((N, 6, 2))
    Ds = np.zeros((N, 6))
    for i in range(6):
        b, a = coeffs[i]
        b0 = b[:, 0]
        a1, a2 = a[:, 1], a[:, 2]
        Ai[:, i, 0, 0] = -a1
        Ai[:, i, 0, 1] = 1.0
        Ai[:, i, 1, 0] = -a2
        Bi[:, i, 0] = b[:, 1] - a1 * b0
        Bi[:, i, 1] = b[:, 2] - a2 * b0
        Ds[:, i] = b0
    for i in range(6):
        ux[:, i + 1] = Ds[:, i] * ux[:, i]
        us[:, i + 1] = Ds[:, i, None] * us[:, i]
        us[:, i + 1, 2 * i] += 1.0
    for i in range(6):
        Ab[:, 2 * i:2 * i + 2, :] = Bi[:, i, :, None] * us[:, None, i, :]
        Ab[:, 2 * i:2 * i + 2, 2 * i:2 * i + 2] += Ai[:, i]
        Bb[:, 2 * i:2 * i + 2] = Bi[:, i] * ux[:, i, None]
    Cb = us[:, 6]
    Db = ux[:, 6]
    Bb = Bb * gain[:, None]
    Db = Db * gain

    h = np.zeros((N, L))
    U = np.zeros((N, L, 12))
    Wm = np.zeros((N, 12, L))
    h[:, 0] = Db
    s = Bb.copy()
    At = np.broadcast_to(np.eye(12), (N, 12, 12)).copy()
    for t in range(L):
        U[:, t, :] = np.einsum('nd,ndk->nk', Cb, At)
        if t >= 1:
            h[:, t] = np.einsum('nd,nd->n', Cb, s)
            s = np.einsum('nij,nj->ni', Ab, s)
        At = np.einsum('nij,njk->nik', Ab, At)
    M = At
    ApB = Bb.copy()
    for i in range(L - 1, -1, -1):
        Wm[:, :, i] = ApB
        ApB = np.einsum('nij,nj->ni', Ab, ApB)

    def g_and_next(Mloc):
        Mp = [np.broadcast_to(np.eye(12), Mloc.shape).copy()]
        for _ in range(1, 8):
            Mp.append(np.einsum('nij,njk->nik', Mloc, Mp[-1]))
        G = np.zeros((N, 96, 96))
        for j in range(8):
            for jp in range(j + 1):
                G[:, 12 * j:12 * j + 12, 12 * jp:12 * jp + 12] = Mp[j - jp]
        Mnext = np.einsum('nij,njk->nik', Mloc, Mp[-1])
        P = np.concatenate(Mp, axis=1)
        return G, P, Mnext

    G1, PD1, M8 = g_and_next(M)
    G2, PD2, M64 = g_and_next(M8)
    G3, PD3, _ = g_and_next(M64)

    Htop = np.zeros((N, L, L), dtype=np.float32)
    for i in range(L):
        Htop[:, i, i:] = h[:, :L - i].astype(np.float32)

    thr = _dn(p, 19, -60.0, 0.0)
    ratio = _dn(p, 20, 1.0, 10.0)
    atk = _dn(p, 21, 1.0, 1000.0)
    rel = _dn(p, 22, 1.0, 1000.0)
    knee = _dn(p, 23, 3.0, 24.0)
    makeup = _dn(p, 24, 0.0, 24.0)
    pan = p[..., 25].astype(np.float64).reshape(-1)
    a_a = np.exp(-1.0 / (SR * atk * 1e-3))
    a_r = np.exp(-1.0 / (SR * rel * 1e-3))
    c = 1.0 - 1.0 / ratio
    cc2 = np.sqrt(c / (2.0 * knee))
    dal = a_a - a_r
    dlna = np.log(a_a) - np.log(a_r)
    peB = np.where(np.abs(dal) > 1e-12, dlna / np.where(dal == 0, 1.0, dal), 0.0)
    peA = K * np.log(a_r) - peB * K * a_r

    cc = np.zeros((N, 16))
    cc[:, 0] = cc2 * (-thr + knee / 2.0)      # bias_u
    cc[:, 1] = cc2 * C1 * 0.5                  # scale_u (ln(y^2)/2)
    cc[:, 2] = cc2 * knee                      # clamp limit
    cc[:, 3] = -thr - knee / 2.0               # bias_rv
    cc[:, 4] = c                               # ratio coefficient
    cc[:, 5] = a_r                             # alpha bias
    cc[:, 6] = dal                             # alpha scale
    cc[:, 7] = makeup * LN10_20                # exp bias
    cc[:, 8] = EPS * EPS                       # ln bias (on y^2)
    cc[:, 9] = peB
    cc[:, 10] = peA
    cc[:, 11] = dlna                           # pe scale from count
    cc[:, 12] = K * np.log(a_r)                # pe bias from count
    ccdev = np.broadcast_to(cc[:, None, :].astype(np.float32), (N, 128, 16)).copy()

    import ml_dtypes
    bf = ml_dtypes.bfloat16
    dtb = bf if TOEP_BF else np.float32
    # [128, NC128] pack per channel: hmov (128 cols) + wt (12 cols, unused)
    pk128 = np.zeros((N, 128, NC128), dtype=dtb)
    pk128[:, :, 0:128] = Htop.astype(dtb)

    # [12, NC12] f32 pack: g2t g3t pd1t pd2t pd3t (g1t folded into wg)
    pk12 = np.concatenate([
        _pack_g(G2), _pack_g(G3),
        np.ascontiguousarray(PD1.transpose(0, 2, 1)).astype(np.float32),
        np.ascontiguousarray(PD2.transpose(0, 2, 1)).astype(np.float32),
        np.ascontiguousarray(PD3.transpose(0, 2, 1)).astype(np.float32),
    ], axis=2)
    # WG_j = wt @ g1t_j : folds the forcing-state matmul into the L1 upsweep.
    WmT = np.transpose(Wm, (0, 2, 1))  # [N,128,12] f64
    wg = np.concatenate(
        [np.einsum('nid,nsd->nis', WmT, G1[:, :, 12 * j:12 * (j + 1)])
         for j in range(8)], axis=2).astype(np.float32)  # [N,128,768]
    ut = np.ascontiguousarray(U.transpose(0, 2, 1)).astype(np.float32)

    consts = dict(pk128=pk128, pk12=pk12, ut=ut, cc=ccdev, wg=wg)
    return consts, dict(pan=pan)


def _pack_g(G):
    # [N,96,96] -> [N,12,768]: block j (cols 12j:12j+12) transposed, stacked on free
    blocks = [np.ascontiguousarray(G[:, :, 12 * j:12 * (j + 1)].transpose(0, 2, 1))
              for j in range(8)]
    return np.concatenate(blocks, axis=2).astype(np.float32)


_CACHE = {}


def _build_graph():
    if "nc" in _CACHE:
        return _CACHE["nc"]
    import concourse.bacc as bacc
    import concourse.mybir as mybir
    from concourse.tile import TileContext

    f32 = mybir.dt.float32
    bf16 = mybir.dt.bfloat16
    Alu = mybir.AluOpType
    Act = mybir.ActivationFunctionType

    from concourse import hw_specs as _hw
    _cached = _hw.get_activation_tables

    def _pinned_tables(arch):
        t = dict(_cached(arch)) if not isinstance(_cached, dict) else dict(_cached)
        k = "natural_log_exp_and_others"
        if k in t:
            return {name: (fns if name == k else set())
                    for name, fns in t.items()}
        return t
    import concourse.bacc as _bacc_mod
    if not os.environ.get("KNOPIN"):
        _bacc_mod.get_activation_tables = _pinned_tables

    nc = bacc.Bacc("TRN2", target_bir_lowering=False, debug=False,
                   num_devices=NCORES)
    dtb = bf16 if TOEP_BF else f32
    xt_d = nc.dram_tensor("xt", [CH, L, K], f32, kind="ExternalInput").ap()
    if TOEP_BF:
        xtb_d = nc.dram_tensor("xtb", [CH, L, K], bf16, kind="ExternalInput").ap()
    pk128_d = nc.dram_tensor("pk128", [128, CH * NC128], dtb,
                             kind="ExternalInput").ap()
    pk12_d = nc.dram_tensor("pk12", [CH, 12, NC12], f32,
                            kind="ExternalInput").ap()
    ut_d = nc.dram_tensor("ut", [12, CH * 128], f32, kind="ExternalInput").ap()
    wg_d = nc.dram_tensor("wg", [CH, 128, 768], f32, kind="ExternalInput").ap()
    cc_d = nc.dram_tensor("cc", [128, CH * 16], f32, kind="ExternalInput").ap()
    id_d = nc.dram_tensor("ident", [128, 128], f32, kind="ExternalInput").ap()
    out_d = nc.dram_tensor("out", [CH, S], bf16, kind="ExternalOutput").ap()
    out_v = out_d.rearrange("c (p f) -> c p f", p=128)
    KDEBUG = bool(os.environ.get("KDEBUG"))
    if KDEBUG:
        dbgy_d = nc.dram_tensor("dbg_y", [CH, S], f32, kind="ExternalOutput").ap()
        dbgy_v = dbgy_d.rearrange("c (p f) -> c p f", p=128)
        dbgg_d = nc.dram_tensor("dbg_g", [CH, S], f32, kind="ExternalOutput").ap()
        dbgg_v = dbgg_d.rearrange("c (p f) -> c p f", p=128)
        dbgd_d = nc.dram_tensor("dbg_d", [CH, S], f32, kind="ExternalOutput").ap()
        dbgd_v = dbgd_d.rearrange("c (p f) -> c p f", p=128)

    import contextlib
    with contextlib.ExitStack() as _stk:
        tc = _stk.enter_context(TileContext(nc))
        _p = lambda *a, **k: _stk.enter_context(tc.tile_pool(*a, **k))
        one_pool = _p(name="one", bufs=1)
        pk_pool = _p(name="pk", bufs=2)
        xin_pool = _p(name="xin", bufs=3)
        xbf_pool = _p(name="xbf", bufs=4)
        sml_pool = _p(name="sml", bufs=3)
        med_pool = _p(name="med", bufs=3)
        ysb_pool = _p(name="ysb", bufs=5)
        gg_pool = _p(name="gg", bufs=4)
        dg_pool = _p(name="dgp", bufs=4)
        dl_pool = _p(name="dl", bufs=3)
        al_pool = _p(name="al", bufs=3)
        tmp_pool = _p(name="tmp", bufs=2)
        tmb_pool = _p(name="tmb", bufs=3)
        py_pool = _p(name="py", bufs=2, space="PSUM")
        ps_pool = _p(name="ps", bufs=2, space="PSUM")
        pf_pool = _p(name="pf", bufs=1, space="PSUM")
        pt_pool = _p(name="pt", bufs=3, space="PSUM")
        if True:
            ident = one_pool.tile([128, 128], f32, tag="ident")
            pk128 = one_pool.tile([128, CH * NC128], dtb, tag="pk128")
            utc = one_pool.tile([12, CH * 128], f32, tag="utc")
            ccc = one_pool.tile([128, CH * 16], f32, tag="ccc")
            zbig = one_pool.tile([128, K], f32, tag="zbig")
            nc.vector.memset(zbig[:], 0.0)

            def CC(ch, i):
                return ccc[:, 16 * ch + i:16 * ch + i + 1]

            def hmov(ch):
                return pk128[:, NC128 * ch:NC128 * ch + 128]

            def g123(ch, lvl, r):  # lvl 1,2 ; block r (96 cols)
                base = 768 * (lvl - 1)
                t = st[ch]["pk12"]
                return t[:, base + 96 * r:base + 96 * (r + 1)]

            def pdt(ch, lvl):  # lvl 0,1,2 -> pd1t,pd2t,pd3t [12,96]
                t = st[ch]["pk12"]
                return t[:, 1536 + 96 * lvl:1536 + 96 * (lvl + 1)]

            def ut_ch(ch):
                return utc[:, 128 * ch:128 * (ch + 1)]

            st = {}

            def s_load(ch):
                d = st[ch] = {}
                d["xt"] = xin_pool.tile([L, K], f32, tag="xt", name=f"xt{ch}")
                d["wg"] = pk_pool.tile([128, 768], f32, tag="wg",
                                       name=f"wg{ch}")
                d["pk12"] = pk_pool.tile([12, NC12], f32, tag="pk12",
                                         name=f"pk12_{ch}")
                nc.sync.dma_start(out=d["xt"][:, 0:512], in_=xt_d[ch][:, 0:512])
                nc.sync.dma_start(out=d["wg"][:, 0:384], in_=wg_d[ch][:, 0:384])
                nc.sync.dma_start(out=d["xt"][:, 512:K], in_=xt_d[ch][:, 512:K])
                nc.sync.dma_start(out=d["wg"][:, 384:768], in_=wg_d[ch][:, 384:768])
                nc.sync.dma_start(out=d["pk12"][:], in_=pk12_d[ch])
                if TOEP_BF:
                    d["xtb"] = xbf_pool.tile([L, K], bf16, tag="xtb",
                                             name=f"xtb{ch}")
                    nc.sync.dma_start(out=d["xtb"][:], in_=xtb_d[ch])
                yield

            def s_upsweep(ch):
                d = st[ch]
                S0T = ps_pool.tile([128, 96], f32, tag="ps")
                for j in range(8):
                    nc.tensor.matmul(
                        out=S0T[:], lhsT=d["xt"][:, 128 * j:128 * (j + 1)],
                        rhs=d["wg"][:, 96 * j:96 * (j + 1)],
                        start=(j == 0), stop=(j == 7))
                d["S0sb"] = med_pool.tile([128, 96], f32, tag="S0sb", name=f"S0sb{ch}")
                nc.scalar.activation(d["S0sb"][:], S0T[:], Act.Copy)
                yield
                z1p = pt_pool.tile([12, 128], f32, tag="pt")
                nc.tensor.transpose(z1p[:], d["S0sb"][:, 84:96], ident[:])
                z1s = sml_pool.tile([12, 128], f32, tag="z1s")
                nc.vector.tensor_copy(z1s[:], z1p[:])
                yield
                S2T = ps_pool.tile([16, 96], f32, tag="ps")
                for r in range(8):
                    nc.tensor.matmul(
                        out=S2T[:], lhsT=z1s[:, r:128:8],
                        rhs=g123(ch, 1, r), start=(r == 0), stop=(r == 7))
                d["S2sb"] = sml_pool.tile([16, 96], f32, tag="S2sb", name=f"S2sb{ch}")
                nc.vector.tensor_copy(d["S2sb"][:], S2T[:])
                yield
                z2p = pt_pool.tile([12, 16], f32, tag="pt")
                nc.tensor.transpose(z2p[:], d["S2sb"][:, 84:96], ident[0:16, 0:16])
                z2s = sml_pool.tile([12, 16], f32, tag="z2s")
                nc.vector.tensor_copy(z2s[:], z2p[:])
                yield
                S3T = ps_pool.tile([2, 96], f32, tag="ps")
                for r in range(8):
                    nc.tensor.matmul(
                        out=S3T[:], lhsT=z2s[:, r:16:8],
                        rhs=g123(ch, 2, r), start=(r == 0), stop=(r == 7))
                d["S3sb"] = sml_pool.tile([2, 96], f32, tag="S3sb", name=f"S3sb{ch}")
                nc.vector.tensor_copy(d["S3sb"][:], S3T[:])
                yield
                z3p = pt_pool.tile([12, 2], f32, tag="pt")
                nc.tensor.transpose(z3p[:], d["S3sb"][:, 84:96], ident[0:2, 0:2])
                d["z3s"] = sml_pool.tile([12, 2], f32, tag="z3s", name=f"z3s{ch}")
                nc.vector.tensor_copy(d["z3s"][:], z3p[:])

            def s_downsweep(ch):
                d = st[ch]
                v3 = sml_pool.tile([12, 2], f32, tag="v3")
                nc.vector.memset(v3[:], 0.0)
                nc.vector.tensor_copy(v3[:, 1:2], d["z3s"][:, 0:1])
                VL3p = ps_pool.tile([2, 96], f32, tag="ps")
                nc.tensor.matmul(out=VL3p[:], lhsT=v3[:], rhs=pdt(ch, 2),
                                 start=True, stop=True)
                WT3 = sml_pool.tile([2, 96], f32, tag="WT3")
                nc.vector.tensor_copy(WT3[:, 0:12], VL3p[:, 0:12])
                nc.vector.tensor_tensor(out=WT3[:, 12:96], in0=VL3p[:, 12:96],
                                        in1=d["S3sb"][:, 0:84], op=Alu.add)
                yield
                vin3 = sml_pool.tile([12, 16], f32, tag="vin3")
                tp3 = pt_pool.tile([12, 16], f32, tag="pt")
                for r in range(8):
                    nc.tensor.transpose(tp3[:, 2 * r:2 * r + 2],
                                        WT3[:, 12 * r:12 * (r + 1)],
                                        ident[0:2, 0:2])
                nc.vector.tensor_copy(
                    vin3.rearrange("p (b r) -> p r b", b=2),
                    tp3.rearrange("p (r b) -> p r b", r=8))
                yield
                VL2p = ps_pool.tile([16, 96], f32, tag="ps")
                nc.tensor.matmul(out=VL2p[:], lhsT=vin3[:], rhs=pdt(ch, 1),
                                 start=True, stop=True)
                WT2 = sml_pool.tile([16, 96], f32, tag="WT2")
                nc.vector.tensor_copy(WT2[:, 0:12], VL2p[:, 0:12])
                nc.vector.tensor_tensor(out=WT2[:, 12:96], in0=VL2p[:, 12:96],
                                        in1=d["S2sb"][:, 0:84], op=Alu.add)
                yield
                vin2 = sml_pool.tile([12, 128], f32, tag="vin2")
                tp2 = pt_pool.tile([12, 128], f32, tag="pt")
                for r in range(8):
                    nc.tensor.transpose(tp2[:, 16 * r:16 * (r + 1)],
                                        WT2[:, 12 * r:12 * (r + 1)],
                                        ident[0:16, 0:16])
                    if r == 3:
                        yield
                nc.vector.tensor_copy(
                    vin2.rearrange("p (b r) -> p r b", b=16),
                    tp2.rearrange("p (r b) -> p r b", r=8))
                yield
                VL1p = ps_pool.tile([128, 96], f32, tag="ps")
                nc.tensor.matmul(out=VL1p[:], lhsT=vin2[:], rhs=pdt(ch, 0),
                                 start=True, stop=True)
                WT1 = sml_pool.tile([128, 96], f32, tag="WT1")
                nc.scalar.activation(WT1[:, 0:12], VL1p[:, 0:12], Act.Copy)
                nc.vector.tensor_tensor(out=WT1[:, 12:96], in0=VL1p[:, 12:96],
                                        in1=d["S0sb"][:, 0:84], op=Alu.add)
                yield
                d["VV1f"] = med_pool.tile([12, K], f32, tag="VV1f", name=f"VV1f{ch}")
                for j in range(8):
                    tp = pt_pool.tile([12, 128], f32, tag="pt")
                    nc.tensor.transpose(tp[:], WT1[:, 12 * j:12 * (j + 1)],
                                        ident[:])
                    if j % 2 == 0:
                        nc.vector.tensor_copy(d["VV1f"][:, 128 * j:128 * (j + 1)],
                                              tp[:])
                    else:
                        nc.scalar.activation(d["VV1f"][:, 128 * j:128 * (j + 1)],
                                             tp[:], Act.Copy)
                    if j % 3 == 2:
                        yield

            def s_ymm(ch):
                d = st[ch]
                xtt = d["xtb"] if TOEP_BF else d["xt"]
                d["ysb"] = ysb_pool.tile([128, K], bf16, tag="ysb", name=f"ysb{ch}")
                for h in range(2):
                    yp = py_pool.tile([128, 512], f32, tag="y")
                    for jj in range(4):
                        j = 4 * h + jj
                        nc.tensor.matmul(
                            out=yp[:, 128 * jj:128 * (jj + 1)],
                            lhsT=xtt[:, 128 * j:128 * (j + 1)],
                            rhs=hmov(ch),
                            start=(jj == 0), stop=False, skip_group_check=True)
                    for jj in range(4):
                        j = 4 * h + jj
                        nc.tensor.matmul(
                            out=yp[:, 128 * jj:128 * (jj + 1)],
                            lhsT=d["VV1f"][:, 128 * j:128 * (j + 1)],
                            rhs=ut_ch(ch),
                            start=False, stop=(jj == 3), skip_group_check=True)
                    nc.scalar.activation(d["ysb"][:, 512 * h:512 * (h + 1)],
                                         yp[:], Act.Copy)
                    yield

            def s_gain(ch):
                d = st[ch]
                H = 512
                ya = tmp_pool.tile([128, K], bf16, tag="tA16")
                lnv = tmp_pool.tile([128, K], f32, tag="f4a")
                u = tmb_pool.tile([128, K], f32, tag="f4b")
                q = tmb_pool.tile([128, K], f32, tag="f4b")
                rv = tmb_pool.tile([128, K], f32, tag="f4b")
                g = gg_pool.tile([128, K], f32, tag="g", name=f"g{ch}")
                dg = dg_pool.tile([128, K], f32, tag="dg", name=f"dg{ch}")
                for h in range(2):
                    s = slice(H * h, H * (h + 1))
                    nc.vector.tensor_tensor(out=ya[:, s], in0=d["ysb"][:, s],
                                            in1=d["ysb"][:, s], op=Alu.mult)
                    yield
                    nc.scalar.activation(lnv[:, s], ya[:, s], Act.Ln,
                                         bias=CC(ch, 8))
                    yield
                    nc.scalar.activation(u[:, s], lnv[:, s], Act.Relu,
                                         bias=CC(ch, 0), scale=CC(ch, 1))
                    yield
                    if h == 0 or ch >= 3:
                        nc.gpsimd.tensor_scalar(out=u[:, s], in0=u[:, s],
                                                scalar1=CC(ch, 2), scalar2=None,
                                                op0=Alu.min)
                    else:
                        nc.vector.tensor_scalar(out=u[:, s], in0=u[:, s],
                                                scalar1=CC(ch, 2), scalar2=None,
                                                op0=Alu.min)
                    yield
                    if h == 0:
                        nc.gpsimd.tensor_tensor(out=q[:, s], in0=u[:, s],
                                                in1=u[:, s], op=Alu.mult)
                    else:
                        nc.scalar.activation(q[:, s], u[:, s], Act.Square)
                    yield
                    nc.scalar.activation(rv[:, s], lnv[:, s], Act.Relu,
                                         bias=CC(ch, 3), scale=C1 * 0.5)
                    yield
                    nc.vector.scalar_tensor_tensor(out=g[:, s], in0=rv[:, s],
                                                   scalar=CC(ch, 4),
                                                   in1=q[:, s],
                                                   op0=Alu.mult, op1=Alu.add)
                    yield
                    if h == 0:
                        nc.gpsimd.tensor_tensor(out=dg[:, 1:H], in0=g[:, 0:H - 1],
                                                in1=g[:, 1:H], op=Alu.subtract)
                        nc.gpsimd.tensor_scalar_mul(dg[:, 0:1], g[:, 0:1], -1.0)
                    elif ch >= 3:
                        nc.gpsimd.tensor_tensor(out=dg[:, H:K], in0=g[:, H - 1:K - 1],
                                                in1=g[:, H:K], op=Alu.subtract)
                    else:
                        nc.vector.tensor_tensor(out=dg[:, H:K], in0=g[:, H - 1:K - 1],
                                                in1=g[:, H:K], op=Alu.subtract)
                    yield
                d["g"] = g
                d["dg"] = dg
                d["delta"] = None

            def s_pass(ch, idx):
                d = st[ch]
                H = 512
                g, dg = d["g"], d["dg"]
                src_ = dg if d["delta"] is None else d["delta"]
                m = tmp_pool.tile([128, K], bf16, tag="m")
                alpha = al_pool.tile([128, K], f32, tag="alpha", bufs=4,
                                     name=f"al{ch}_{idx}")
                dzs = tmp_pool.tile([128, K], f32, tag="dzs")
                stg = sml_pool.tile([128, 2], f32, tag="stg")
                cnt = sml_pool.tile([128, 2], f32, tag="sacc")
                for h in range(2):
                    s = slice(H * h, H * (h + 1))
                    if h == 0:
                        nc.gpsimd.tensor_scalar(out=m[:, s], in0=src_[:, s],
                                                scalar1=0.0, scalar2=None,
                                                op0=Alu.is_lt)
                        if PE_MCNT:
                            nc.vector.tensor_scalar(out=m[:, s], in0=m[:, s],
                                                    scalar1=1.0, scalar2=0.0,
                                                    op0=Alu.mult, op1=Alu.add,
                                                    accum_out=cnt[:, h:h + 1])
                    else:
                        if PE_MCNT:
                            nc.vector.tensor_scalar(out=m[:, s], in0=src_[:, s],
                                                    scalar1=0.0, scalar2=0.0,
                                                    op0=Alu.is_lt, op1=Alu.add,
                                                    accum_out=cnt[:, h:h + 1])
                        else:
                            nc.gpsimd.tensor_scalar(out=m[:, s], in0=src_[:, s],
                                                    scalar1=0.0, scalar2=None,
                                                    op0=Alu.is_lt)
                    yield
                    nc.scalar.activation(alpha[:, s], m[:, s], Act.Identity,
                                         bias=CC(ch, 5), scale=CC(ch, 6))
                    yield
                    nc.vector.tensor_tensor_scan(
                        out=dzs[:, s], data0=dg[:, s], data1=alpha[:, s],
                        initial=(0.0 if h == 0 else dzs[:, H - 1:H]),
                        op0=Alu.add, op1=Alu.mult)
                    yield
                if PE_MCNT:
                    nc.vector.tensor_tensor(out=cnt[:, 0:1], in0=cnt[:, 0:1],
                                            in1=cnt[:, 1:2], op=Alu.add)
                    cntp = ps_pool.tile([1, 128], f32, tag="ps")
                    nc.tensor.transpose(cntp[:], cnt[:, 0:1], ident[:])
                    pesb = sml_pool.tile([1, 128], f32, tag="pesb")
                    nc.scalar.activation(pesb[:], cntp[:], Act.Exp,
                                         bias=CC(ch, 12)[0:1],
                                         scale=CC(ch, 11)[0:1])
                nc.vector.tensor_tensor(out=stg[:, 0:1], in0=dzs[:, K - 1:K],
                                        in1=g[:, K - 1:K], op=Alu.add)
                tp1 = ps_pool.tile([1, 128], f32, tag="ps")
                nc.tensor.transpose(tp1[:], stg[:, 0:1], ident[:])
                chs = sml_pool.tile([1, 129], f32, tag="chs")
                nc.vector.memset(chs[:, 0:1], 0.0)
                nc.vector.tensor_tensor_scan(out=chs[:, 1:129], data0=pesb[:],
                                             data1=tp1[:], initial=0.0,
                                             op0=Alu.mult, op1=Alu.add)
                icp = pt_pool.tile([128, 1], f32, tag="pt")
                nc.tensor.transpose(icp[:], chs[:, 0:128], ident[0:1, 0:1])
                yield
                ndelta = dl_pool.tile([128, K], f32, tag=f"d{idx % 2}",
                                      name=f"dl{ch}_{idx}")
                for h in range(2):
                    s = slice(H * h, H * (h + 1))
                    nc.vector.tensor_tensor_scan(
                        out=ndelta[:, s], data0=dg[:, s], data1=alpha[:, s],
                        initial=(icp[:] if h == 0 else ndelta[:, H - 1:H]),
                        op0=Alu.add, op1=Alu.mult)
                    yield
                d["delta"] = ndelta

            def s_out(ch):
                d = st[ch]
                H = 256 if ch >= 6 else 512
                e = tmb_pool.tile([128, K], f32, tag="f4b")
                expg = tmp_pool.tile([128, K], bf16, tag="tC16")
                z = tmp_pool.tile([128, K], bf16, tag="z16")
                for h in range(K // H):
                    s = slice(H * h, H * (h + 1))
                    if h == 0 or ch >= 3:
                        nc.gpsimd.tensor_tensor(out=e[:, s], in0=d["g"][:, s],
                                                in1=d["delta"][:, s], op=Alu.add)
                    else:
                        nc.vector.tensor_tensor(out=e[:, s], in0=d["g"][:, s],
                                                in1=d["delta"][:, s], op=Alu.add)
                    yield
                    nc.scalar.activation(expg[:, s], e[:, s], Act.Exp,
                                         bias=CC(ch, 7), scale=-LN10_20)
                    yield
                    nc.vector.tensor_tensor(out=z[:, s], in0=d["ysb"][:, s],
                                            in1=expg[:, s], op=Alu.mult)
                    nc.sync.dma_start(out=out_v[ch][:, s], in_=z[:, s])
                    yield
                if KDEBUG:
                    yf = tmp_pool.tile([128, K], f32, tag="dbgy")
                    nc.scalar.activation(yf[:], d["ysb"][:], Act.Copy)
                    nc.sync.dma_start(out=dbgy_v[ch], in_=yf[:])
                    nc.sync.dma_start(out=dbgg_v[ch], in_=d["g"][:])
                    nc.sync.dma_start(out=dbgd_v[ch], in_=d["delta"][:])
                st[ch] = None

            stages = [s_load, s_upsweep, s_downsweep, s_ymm,
                      s_gain]
            stages += [(lambda c, i=i: s_pass(c, i)) for i in range(NPASS)]
            stages += [s_out]

            def chan_prog(ch):
                for stf in stages:
                    r = stf(ch)
                    if r is not None:
                        yield from r

            DSTAG = int(os.environ.get("KSTAG", "8"))
            TAILC = int(os.environ.get("KTAIL", "0"))
            _stv = os.environ.get("KSTV", "0,6,12,19,28,36,47,55")
            if _stv:
                starts = [int(x) for x in _stv.split(",")]
            else:
                starts = [min(ch, 6) * DSTAG + max(ch - 6, 0) * (DSTAG - TAILC)
                          for ch in range(CH)]
            KORD = os.environ.get("KORD", "fwd")
            KADV = int(os.environ.get("KADV", "1"))
            gens = {ch: chan_prog(ch) for ch in range(CH)}
            ticks = 0
            live = list(range(CH))
            _kp = os.environ.get("KPERM", "")
            PERM = [int(x) for x in _kp.split(",")] if _kp else list(range(CH))
            _deferred = [False]
            while live:
                if ticks == 1 and not _deferred[0]:
                    nc.sync.dma_start(out=ident[:], in_=id_d)
                    nc.sync.dma_start(out=pk128[:], in_=pk128_d)
                    nc.sync.dma_start(out=utc[:], in_=ut_d)
                    nc.sync.dma_start(out=ccc[:], in_=cc_d)
                    _deferred[0] = True
                order = [c for c in PERM if c in live]
                if KORD == "pp" and ticks % 2 == 1:
                    order = order[::-1]
                elif KORD == "rot":
                    k = ticks % max(len(order), 1)
                    order = order[k:] + order[:k]
                for ch in order:
                    if ticks < starts[ch]:
                        continue
                    for _ in range(KADV):
                        try:
                            next(gens[ch])
                        except StopIteration:
                            if ch in live:
                                live.remove(ch)
                            break
                ticks += 1

    nc.compile()
    _CACHE["nc"] = nc
    return nc


def _prep_inputs(tracks, mix_params):
    import ml_dtypes
    bf = ml_dtypes.bfloat16
    B, T, _ = tracks.shape
    N = B * T
    x = np.ascontiguousarray(tracks.reshape(N, S)).astype(np.float32)
    consts, host = _host_constants(np.asarray(mix_params))
    xx = x.reshape(N, K, L)
    xt = np.ascontiguousarray(
        xx.reshape(N, 128, 8, L).transpose(0, 3, 2, 1)).reshape(N, L, K)
    ident = np.eye(128, dtype=np.float32)
    in_maps = []
    for c in range(NCORES):
        sl = slice(c * CH, (c + 1) * CH)
        im = {
            "xt": np.ascontiguousarray(xt[sl]),
            "wg": np.ascontiguousarray(consts["wg"][sl]),
            "pk128": np.ascontiguousarray(
                consts["pk128"][sl].transpose(1, 0, 2)).reshape(128, CH * NC128),
            "pk12": np.ascontiguousarray(consts["pk12"][sl]),
            "ut": np.ascontiguousarray(
                consts["ut"][sl].transpose(1, 0, 2)).reshape(12, CH * 128),

            "cc": np.ascontiguousarray(
                consts["cc"][sl].transpose(1, 0, 2)).reshape(128, CH * 16),
            "ident": ident,
        }
        if TOEP_BF:
            im["xtb"] = np.ascontiguousarray(xt[sl]).astype(bf)
        in_maps.append(im)
    return in_maps, host


def kernel(tracks, mix_params):
    from concourse.bass_utils import run_bass_kernel_spmd

    B, T, _ = tracks.shape
    in_maps, host = _prep_inputs(tracks, mix_params)
    nc = _build_graph()
    res = run_bass_kernel_spmd(nc, in_maps, core_ids=list(range(NCORES)))
    zs = np.concatenate([np.asarray(res.results[c]["out"]).astype(np.float32)
                         for c in range(NCORES)], axis=0)

    theta = host["pan"] * (np.pi / 2.0)
    cw = np.cos(theta).astype(np.float32).reshape(B, T, 1)
    sw = np.sin(theta).astype(np.float32).reshape(B, T, 1)
    zb = zs.reshape(B, T, S)
    left = (cw * zb).sum(axis=1)
    right = (sw * zb).sum(axis=1)
    return np.stack([left, right], axis=1).astype(np.float32)
